# revision 8
# baseline (speedup 1.0000x reference)
"""Trainium2 Bass kernel for 2D single-level DWT (coif1, symmetric padding).

Input  x: (4, 64, 512, 512) fp32
Output  : (4, 256, 258, 258) fp32  -- per input channel: [cA, cH, cV, cD]

v2 design (bf16, banded half-blocks, SWDGE stores):
  pass 1 (contract rows r): r is split into half-blocks h in {0,1} of 256
    contiguous rows, each loaded as partition p <- rows (256h + 2p + j),
    j in {0,1} -- every DMA descriptor is 2 contiguous DRAM rows (2 KB).
    The 6-tap band of R_f means half-block h only feeds kh in
    [128h, 128h+130), so each matmul streams only 130 columns; the 2-col
    overlap accumulates via PSUM has_written semantics.
      Yt_f[c, kh] = sum_r X[r, c] R_f[kh, r]
  pass 2 (contract cols c): output rows (g, kw) are packed in uniform
    128-partition chunks of 64 kw x {lo,hi}: chunks start at kw =
    0, 64, 128, 192, 194 (the last overlaps; host keeps only kw 256-257
    from it).  Each chunk's band covers at most 2 c-blocks of 128 -> 1-2
    accumulating matmuls of 258 columns.
      O_{f,g}[kw, kh] = sum_c R_g[kw, c] Yt_f[c, kh]
  All matmuls bf16 (1 cycle/row at any free size, FWL weight loads).
  Loads and stores ride the gpsimd SWDGE ring: HWDGE stores with <128
  partitions serialize onto 2 of 16 SDMA engines; SWDGE spreads all 16.
  Output leaves packed ([i, p, block, kh], bf16); the host unpacks.
"""

import os
import sys

for _p in ("/opt/trn_rl_repo", "/opt/pypackages"):
    if _p not in sys.path:
        sys.path.append(_p)

os.environ.setdefault("JAX_COMPILATION_CACHE_DIR", "/tmp/jax_comp_cache")
os.environ.setdefault("JAX_PERSISTENT_CACHE_MIN_COMPILE_TIME_SECS", "10")

import numpy as np
import ml_dtypes

import concourse.bass as bass
import concourse.bacc as bacc
import concourse.mybir as mybir
from concourse.bass_utils import run_bass_kernel_spmd
from concourse.tile import TileContext

N_CORES = 8
H = W = 512
OUT = 258
IMGS = 32  # images per core (4*64/8)
F32 = mybir.dt.float32
BF16 = mybir.dt.bfloat16
NPBF16 = ml_dtypes.bfloat16

# pywt coif1 decomposition filters, flipped to correlation form
DEC_LO = np.array([-0.01565572813546454, -0.0727326195128539, 0.38486484686420286,
                   0.8525720202122554, 0.3378976624578092, -0.0727326195128539])
DEC_HI = np.array([0.0727326195128539, 0.3378976624578092, -0.8525720202122554,
                   0.38486484686420286, 0.0727326195128539, -0.01565572813546454])
FLEN = 6
PAD = 4
LO_F = DEC_LO[::-1]
HI_F = DEC_HI[::-1]

# pass-2 packed chunks: 64 kw starting at KWSTART[C]; CHUNK_CC[C] = c-blocks
KWSTART = [0, 64, 128, 192, 194]
CHUNK_CC = [[0], [0, 1], [1, 2], [2, 3], [3]]
W2_SLICES = [(C, cc) for C in range(5) for cc in CHUNK_CC[C]]  # 8 slices
# STG block order = device copy order (tA_f0, tB_f0, tA_f1, tB_f1, t5)
STG_BLOCKS = [(0, 0), (0, 1), (0, 2), (0, 3),
              (1, 0), (1, 1), (1, 2), (1, 3),
              (0, 4), (1, 4)]


def _build_R(filt: np.ndarray, n: int = W) -> np.ndarray:
    """Banded [258, 512] operator: out[k] = sum_j filt[j] * x[sym(2k + j - PAD)]."""
    out_len = (n + FLEN - 1) // 2

    def sym(i: int) -> int:
        while i < 0 or i >= n:
            if i < 0:
                i = -i - 1
            if i >= n:
                i = 2 * n - 1 - i
        return i

    R = np.zeros((out_len, n), dtype=np.float64)
    for k in range(out_len):
        for j in range(FLEN):
            R[k, sym(2 * k + j - PAD)] += filt[j]
    return R


_R = [_build_R(LO_F), _build_R(HI_F)]


def _build_w1() -> np.ndarray:
    """w1[p, ((f*2+h)*2+j)*130 + t] = R_f[128h + t, 256h + 2p + j]."""
    w = np.zeros((128, 8, 130), np.float64)
    for f in range(2):
        for h in range(2):
            for j in range(2):
                rows = 256 * h + 2 * np.arange(128) + j
                khs = 128 * h + np.arange(130)
                w[:, (f * 2 + h) * 2 + j, :] = _R[f][np.ix_(khs, rows)].T
    return w.reshape(128, 8 * 130).astype(NPBF16)


def _build_w2() -> np.ndarray:
    """w2[p, s*128 + u] for slice s=(C, cc): R_{u//64}[KWSTART[C] + u%64, 128cc + p]."""
    cols = []
    for C, cc in W2_SLICES:
        w = np.zeros((128, 128), np.float64)
        for u in range(128):
            g, kwo = divmod(u, 64)
            w[:, u] = _R[g][KWSTART[C] + kwo, cc * 128:(cc + 1) * 128]
        cols.append(w)
    return np.concatenate(cols, axis=1).astype(NPBF16)


_W1 = _build_w1()
_W2 = _build_w2()
_MODULE = None
PS_BUFS = 4
X_BUFS = 3
YT_BUFS = 2
STG_BUFS = 2


def _build_module() -> bass.Bass:
    nc = bacc.Bacc("TRN2", target_bir_lowering=False, debug=False)
    x_in = nc.declare_dram_parameter("x", [IMGS, H, W], BF16, isOutput=False)
    w1_in = nc.declare_dram_parameter("w1", [128, 8 * 130], BF16, isOutput=False)
    w2_in = nc.declare_dram_parameter("w2", [128, 8 * 128], BF16, isOutput=False)
    y_out = nc.declare_dram_parameter("y", [IMGS, 128, 10 * OUT], BF16, isOutput=True)

    with TileContext(nc) as tc:
        with (
            tc.tile_pool(name="wpool", bufs=1) as wpool,
            tc.tile_pool(name="xpool", bufs=X_BUFS) as xpool,
            tc.tile_pool(name="ypool", bufs=YT_BUFS) as ypool,
            tc.tile_pool(name="spool", bufs=STG_BUFS) as spool,
            tc.tile_pool(name="psum", bufs=2, space="PSUM") as pspool,
        ):
            # First image load goes on the (otherwise idle) sync ring, ahead
            # of the weight DMAs: HWDGE first-byte latency ~0.6us vs the
            # SWDGE ring's multi-us cold start.
            X0 = xpool.tile([128, 2, 1024], BF16, tag="X", name="X_0")
            nc.sync.dma_start(
                out=X0[:],
                in_=x_in[0].rearrange("(h p j) c -> p h (j c)", h=2, j=2),
            )
            Wt1 = wpool.tile([128, 8 * 130], BF16)
            Wt2 = wpool.tile([128, 8 * 128], BF16)
            nc.sync.dma_start(out=Wt1[:], in_=w1_in[:])
            nc.sync.dma_start(out=Wt2[:], in_=w2_in[:])

            # Tiny PE op consuming both weight DMAs so later matmuls depend
            # on them via PE program order (Matmult carries one sync wait).
            warm = pspool.tile([128, 1024], F32, tag="ps", bufs=PS_BUFS,
                               name="warm")
            nc.tensor.matmul(warm[0:1, 0:OUT], lhsT=Wt2[:, 0:1],
                             rhs=Wt1[:, 0:OUT], start=True, stop=True)

            ev = 0

            def copy(dst, src):
                nonlocal ev
                if ev % 2 == 0:
                    nc.scalar.copy(out=dst, in_=src)
                else:
                    nc.vector.tensor_copy(out=dst, in_=src)
                ev += 1

            def load_x(i):
                # X[p, h, j*512 + c] = x[i, 256h + 2p + j, c]
                X = xpool.tile([128, 2, 1024], BF16, tag="X", name=f"X_{i}")
                nc.gpsimd.dma_start(
                    out=X[:],
                    in_=x_in[i].rearrange("(h p j) c -> p h (j c)", h=2, j=2),
                )
                return X

            Xnext = X0
            for i in range(IMGS):
                Xr = Xnext[:]
                Yt = ypool.tile([128, 8 * OUT], BF16, tag="Yt", name=f"Yt_{i}")

                # pass 1: Yt[p, (f*4+cc)*258 + kh] = Yt_f[c = 128cc + p, kh]
                for ccp in range(2):  # cc pair (2*ccp, 2*ccp+1)
                    ps1 = [pspool.tile([128, 1024], F32, tag="ps", bufs=PS_BUFS,
                                       name=f"ps1_{i}_{ccp}_{f}")
                           for f in range(2)]
                    for ci in range(2):
                        cc = 2 * ccp + ci
                        for h in range(2):
                            for j in range(2):
                                lhsT = Xr[:, h, j * 512 + cc * 128:
                                          j * 512 + cc * 128 + 128]
                                for f in range(2):
                                    nc.tensor.matmul(
                                        ps1[f][:, ci * 512 + 128 * h:
                                               ci * 512 + 128 * h + 130],
                                        lhsT=lhsT,
                                        rhs=Wt1[:, ((f * 2 + h) * 2 + j) * 130:
                                                ((f * 2 + h) * 2 + j + 1) * 130],
                                        start=(h == 0 and j == 0),
                                        stop=(h == 1 and j == 1),
                                    )
                    for f in range(2):
                        src = ps1[f][:].rearrange("p (b x) -> p b x", b=2)[:, :, 0:OUT]
                        dst = Yt[:, (f * 4 + 2 * ccp) * OUT:
                                 (f * 4 + 2 * ccp + 2) * OUT].rearrange(
                                     "p (b k) -> p b k", b=2)
                        copy(dst, src)

                # prefetch next image's input
                if i + 1 < IMGS:
                    Xnext = load_x(i + 1)

                # pass 2: STG blocks per STG_BLOCKS order
                STG = spool.tile([128, 10 * OUT], BF16, tag="STG", name=f"STG_{i}")

                def mm_chunk(dst_ap, f, C):
                    ccs = CHUNK_CC[C]
                    for a, cc in enumerate(ccs):
                        s_idx = W2_SLICES.index((C, cc))
                        nc.tensor.matmul(
                            dst_ap,
                            lhsT=Wt2[:, s_idx * 128:(s_idx + 1) * 128],
                            rhs=Yt[:, (f * 4 + cc) * OUT:(f * 4 + cc + 1) * OUT],
                            start=(a == 0),
                            stop=(a == len(ccs) - 1),
                        )

                for f in range(2):
                    tA = pspool.tile([128, 1024], F32, tag="ps", bufs=PS_BUFS, name=f"tA_{i}_{f}")
                    mm_chunk(tA[:, 0:OUT], f, 0)
                    mm_chunk(tA[:, 512:512 + OUT], f, 1)
                    copy(
                        STG[:, (f * 4) * OUT:(f * 4 + 2) * OUT].rearrange(
                            "p (b k) -> p b k", b=2),
                        tA[:].rearrange("p (b x) -> p b x", b=2)[:, :, 0:OUT],
                    )
                for f in range(2):
                    tB = pspool.tile([128, 1024], F32, tag="ps", bufs=PS_BUFS, name=f"tB_{i}_{f}")
                    mm_chunk(tB[:, 0:OUT], f, 2)
                    mm_chunk(tB[:, 512:512 + OUT], f, 3)
                    copy(
                        STG[:, (f * 4 + 2) * OUT:(f * 4 + 4) * OUT].rearrange(
                            "p (b k) -> p b k", b=2),
                        tB[:].rearrange("p (b x) -> p b x", b=2)[:, :, 0:OUT],
                    )
                t5 = pspool.tile([128, 1024], F32, tag="ps", bufs=PS_BUFS, name=f"t5_{i}")
                mm_chunk(t5[:, 0:OUT], 0, 4)
                mm_chunk(t5[:, 512:512 + OUT], 1, 4)
                copy(
                    STG[:, 8 * OUT:10 * OUT].rearrange("p (b k) -> p b k", b=2),
                    t5[:].rearrange("p (b x) -> p b x", b=2)[:, :, 0:OUT],
                )

                nc.gpsimd.dma_start(out=y_out[i], in_=STG[:])
    nc.finalize()
    return nc


def _get_module() -> bass.Bass:
    global _MODULE
    if _MODULE is None:
        _MODULE = _build_module()
    return _MODULE


def _make_in_maps(x: np.ndarray) -> list:
    imgs = x.reshape(N_CORES * IMGS, H, W).astype(NPBF16)
    return [
        {"x": imgs[k * IMGS:(k + 1) * IMGS], "w1": _W1, "w2": _W2}
        for k in range(N_CORES)
    ]


def _unpack(y: np.ndarray, B: int, C: int) -> np.ndarray:
    """y: [n_imgs, 128, 10*258] bf16 -> [B, 4C, 258, 258] fp32."""
    n = y.shape[0]
    y = y.astype(np.float32).reshape(n, 128, 10, OUT)
    full = np.empty((n, 4, OUT, OUT), np.float32)
    for b, (f, Ck) in enumerate(STG_BLOCKS):
        blk = y[:, :, b, :]  # [n, 128(g,kw), 258(kh)]
        for g in range(2):
            s = f + 2 * g
            sel = blk[:, g * 64:(g + 1) * 64, :]
            kws = KWSTART[Ck] + np.arange(64)
            if Ck == 3:
                pass  # kw 192..255, all valid
            elif Ck == 4:
                sel = sel[:, 62:, :]  # only kw 256, 257
                kws = kws[62:]
            full[:, s, :, kws[0]:kws[-1] + 1] = sel.transpose(0, 2, 1)
    return np.ascontiguousarray(full.reshape(B, 4 * C, OUT, OUT))


def kernel(**inputs) -> np.ndarray:
    x = np.asarray(inputs["x"], dtype=np.float32)
    B, C, Hx, Wx = x.shape
    assert (Hx, Wx) == (H, W) and B * C == N_CORES * IMGS

    nc = _get_module()
    res = run_bass_kernel_spmd(nc, _make_in_maps(x), list(range(N_CORES))).results
    y = np.concatenate([res[k]["y"] for k in range(N_CORES)], axis=0)
    return _unpack(y, B, C)


# revision 9
# speedup vs baseline: 1.0134x; 1.0134x over previous
"""Trainium2 Bass kernel for 2D single-level DWT (coif1, symmetric padding).

Input  x: (4, 64, 512, 512) fp32
Output  : (4, 256, 258, 258) fp32  -- per input channel: [cA, cH, cV, cD]

v2 design (bf16, banded half-blocks, SWDGE stores):
  pass 1 (contract rows r): r is split into half-blocks h in {0,1} of 256
    contiguous rows, each loaded as partition p <- rows (256h + 2p + j),
    j in {0,1} -- every DMA descriptor is 2 contiguous DRAM rows (2 KB).
    The 6-tap band of R_f means half-block h only feeds kh in
    [128h, 128h+130), so each matmul streams only 130 columns; the 2-col
    overlap accumulates via PSUM has_written semantics.
      Yt_f[c, kh] = sum_r X[r, c] R_f[kh, r]
  pass 2 (contract cols c): output rows (g, kw) are packed in uniform
    128-partition chunks of 64 kw x {lo,hi}: chunks start at kw =
    0, 64, 128, 192, 194 (the last overlaps; host keeps only kw 256-257
    from it).  Each chunk's band covers at most 2 c-blocks of 128 -> 1-2
    accumulating matmuls of 258 columns.
      O_{f,g}[kw, kh] = sum_c R_g[kw, c] Yt_f[c, kh]
  All matmuls bf16 (1 cycle/row at any free size, FWL weight loads).
  Loads and stores ride the gpsimd SWDGE ring: HWDGE stores with <128
  partitions serialize onto 2 of 16 SDMA engines; SWDGE spreads all 16.
  Output leaves packed ([i, p, block, kh], bf16); the host unpacks.
"""

import os
import sys

for _p in ("/opt/trn_rl_repo", "/opt/pypackages"):
    if _p not in sys.path:
        sys.path.append(_p)

os.environ.setdefault("JAX_COMPILATION_CACHE_DIR", "/tmp/jax_comp_cache")
os.environ.setdefault("JAX_PERSISTENT_CACHE_MIN_COMPILE_TIME_SECS", "10")

import numpy as np
import ml_dtypes

import concourse.bass as bass
import concourse.bacc as bacc
import concourse.mybir as mybir
from concourse.bass_utils import run_bass_kernel_spmd
from concourse.tile import TileContext

N_CORES = 8
H = W = 512
OUT = 258
IMGS = 32  # images per core (4*64/8)
F32 = mybir.dt.float32
BF16 = mybir.dt.bfloat16
NPBF16 = ml_dtypes.bfloat16

# pywt coif1 decomposition filters, flipped to correlation form
DEC_LO = np.array([-0.01565572813546454, -0.0727326195128539, 0.38486484686420286,
                   0.8525720202122554, 0.3378976624578092, -0.0727326195128539])
DEC_HI = np.array([0.0727326195128539, 0.3378976624578092, -0.8525720202122554,
                   0.38486484686420286, 0.0727326195128539, -0.01565572813546454])
FLEN = 6
PAD = 4
LO_F = DEC_LO[::-1]
HI_F = DEC_HI[::-1]

# pass-2 packed chunks: 64 kw starting at KWSTART[C]; CHUNK_CC[C] = c-blocks
KWSTART = [0, 64, 128, 192, 194]
CHUNK_CC = [[0], [0, 1], [1, 2], [2, 3], [3]]
W2_SLICES = [(C, cc) for C in range(5) for cc in CHUNK_CC[C]]  # 8 slices
# STG block order = device copy order (tA_f0, tB_f0, tA_f1, tB_f1, t5)
STG_BLOCKS = [(0, 0), (0, 1), (0, 2), (0, 3),
              (1, 0), (1, 1), (1, 2), (1, 3),
              (0, 4), (1, 4)]


def _build_R(filt: np.ndarray, n: int = W) -> np.ndarray:
    """Banded [258, 512] operator: out[k] = sum_j filt[j] * x[sym(2k + j - PAD)]."""
    out_len = (n + FLEN - 1) // 2

    def sym(i: int) -> int:
        while i < 0 or i >= n:
            if i < 0:
                i = -i - 1
            if i >= n:
                i = 2 * n - 1 - i
        return i

    R = np.zeros((out_len, n), dtype=np.float64)
    for k in range(out_len):
        for j in range(FLEN):
            R[k, sym(2 * k + j - PAD)] += filt[j]
    return R


_R = [_build_R(LO_F), _build_R(HI_F)]


def _build_w1() -> np.ndarray:
    """w1[p, ((f*2+h)*2+j)*130 + t] = R_f[128h + t, 256h + 2p + j]."""
    w = np.zeros((128, 8, 130), np.float64)
    for f in range(2):
        for h in range(2):
            for j in range(2):
                rows = 256 * h + 2 * np.arange(128) + j
                khs = 128 * h + np.arange(130)
                w[:, (f * 2 + h) * 2 + j, :] = _R[f][np.ix_(khs, rows)].T
    return w.reshape(128, 8 * 130).astype(NPBF16)


def _build_w2() -> np.ndarray:
    """w2[p, s*128 + u] for slice s=(C, cc): R_{u//64}[KWSTART[C] + u%64, 128cc + p]."""
    cols = []
    for C, cc in W2_SLICES:
        w = np.zeros((128, 128), np.float64)
        for u in range(128):
            g, kwo = divmod(u, 64)
            w[:, u] = _R[g][KWSTART[C] + kwo, cc * 128:(cc + 1) * 128]
        cols.append(w)
    return np.concatenate(cols, axis=1).astype(NPBF16)


_W1 = _build_w1()
_W2 = _build_w2()
_MODULE = None
PS_BUFS = 4
X_BUFS = 3
YT_BUFS = 2
STG_BUFS = 2


def _build_module() -> bass.Bass:
    nc = bacc.Bacc("TRN2", target_bir_lowering=False, debug=False)
    x_in = nc.declare_dram_parameter("x", [IMGS, H, W], BF16, isOutput=False)
    w1_in = nc.declare_dram_parameter("w1", [128, 8 * 130], BF16, isOutput=False)
    w2_in = nc.declare_dram_parameter("w2", [128, 8 * 128], BF16, isOutput=False)
    y_out = nc.declare_dram_parameter("y", [IMGS, 128, 10 * OUT], BF16, isOutput=True)

    with TileContext(nc) as tc:
        with (
            tc.tile_pool(name="wpool", bufs=1) as wpool,
            tc.tile_pool(name="xpool", bufs=X_BUFS) as xpool,
            tc.tile_pool(name="ypool", bufs=YT_BUFS) as ypool,
            tc.tile_pool(name="spool", bufs=STG_BUFS) as spool,
            tc.tile_pool(name="psum", bufs=2, space="PSUM") as pspool,
        ):
            # Prologue: weights on the sync ring (small, fast HWDGE gen) while
            # the first image's two row-halves generate in parallel on the
            # scalar HWDGE ring and the gpsimd SWDGE ring.
            Wt1 = wpool.tile([128, 8 * 130], BF16)
            Wt2 = wpool.tile([128, 8 * 128], BF16)
            nc.sync.dma_start(out=Wt1[:], in_=w1_in[:])
            nc.sync.dma_start(out=Wt2[:], in_=w2_in[:])
            X0 = xpool.tile([128, 2, 1024], BF16, tag="X", name="X_0")
            x0v = x_in[0].rearrange("(h p j) c -> p h (j c)", h=2, j=2)
            nc.scalar.dma_start(out=X0[:, 0], in_=x0v[:, 0])
            nc.gpsimd.dma_start(out=X0[:, 1], in_=x0v[:, 1])

            # Tiny PE op consuming both weight DMAs so later matmuls depend
            # on them via PE program order (Matmult carries one sync wait).
            warm = pspool.tile([128, 1024], F32, tag="ps", bufs=PS_BUFS,
                               name="warm")
            nc.tensor.matmul(warm[0:1, 0:OUT], lhsT=Wt2[:, 0:1],
                             rhs=Wt1[:, 0:OUT], start=True, stop=True)

            ev = 0

            def copy(dst, src):
                nonlocal ev
                if ev % 2 == 0:
                    nc.scalar.copy(out=dst, in_=src)
                else:
                    nc.vector.tensor_copy(out=dst, in_=src)
                ev += 1

            def load_x(i):
                # X[p, h, j*512 + c] = x[i, 256h + 2p + j, c]
                X = xpool.tile([128, 2, 1024], BF16, tag="X", name=f"X_{i}")
                nc.gpsimd.dma_start(
                    out=X[:],
                    in_=x_in[i].rearrange("(h p j) c -> p h (j c)", h=2, j=2),
                )
                return X

            Xnext = X0
            for i in range(IMGS):
                Xr = Xnext[:]
                Yt = ypool.tile([128, 8 * OUT], BF16, tag="Yt", name=f"Yt_{i}")

                # pass 1: Yt[p, (f*4+cc)*258 + kh] = Yt_f[c = 128cc + p, kh]
                for ccp in range(2):  # cc pair (2*ccp, 2*ccp+1)
                    ps1 = [pspool.tile([128, 1024], F32, tag="ps", bufs=PS_BUFS,
                                       name=f"ps1_{i}_{ccp}_{f}")
                           for f in range(2)]
                    for ci in range(2):
                        cc = 2 * ccp + ci
                        for h in range(2):
                            for j in range(2):
                                lhsT = Xr[:, h, j * 512 + cc * 128:
                                          j * 512 + cc * 128 + 128]
                                for f in range(2):
                                    nc.tensor.matmul(
                                        ps1[f][:, ci * 512 + 128 * h:
                                               ci * 512 + 128 * h + 130],
                                        lhsT=lhsT,
                                        rhs=Wt1[:, ((f * 2 + h) * 2 + j) * 130:
                                                ((f * 2 + h) * 2 + j + 1) * 130],
                                        start=(h == 0 and j == 0),
                                        stop=(h == 1 and j == 1),
                                    )
                    for f in range(2):
                        src = ps1[f][:].rearrange("p (b x) -> p b x", b=2)[:, :, 0:OUT]
                        dst = Yt[:, (f * 4 + 2 * ccp) * OUT:
                                 (f * 4 + 2 * ccp + 2) * OUT].rearrange(
                                     "p (b k) -> p b k", b=2)
                        copy(dst, src)

                # prefetch next image's input
                if i + 1 < IMGS:
                    Xnext = load_x(i + 1)

                # pass 2: STG blocks per STG_BLOCKS order
                STG = spool.tile([128, 10 * OUT], BF16, tag="STG", name=f"STG_{i}")

                def mm_chunk(dst_ap, f, C):
                    ccs = CHUNK_CC[C]
                    for a, cc in enumerate(ccs):
                        s_idx = W2_SLICES.index((C, cc))
                        nc.tensor.matmul(
                            dst_ap,
                            lhsT=Wt2[:, s_idx * 128:(s_idx + 1) * 128],
                            rhs=Yt[:, (f * 4 + cc) * OUT:(f * 4 + cc + 1) * OUT],
                            start=(a == 0),
                            stop=(a == len(ccs) - 1),
                        )

                for f in range(2):
                    tA = pspool.tile([128, 1024], F32, tag="ps", bufs=PS_BUFS, name=f"tA_{i}_{f}")
                    mm_chunk(tA[:, 0:OUT], f, 0)
                    mm_chunk(tA[:, 512:512 + OUT], f, 1)
                    copy(
                        STG[:, (f * 4) * OUT:(f * 4 + 2) * OUT].rearrange(
                            "p (b k) -> p b k", b=2),
                        tA[:].rearrange("p (b x) -> p b x", b=2)[:, :, 0:OUT],
                    )
                t5 = pspool.tile([128, 1024], F32, tag="ps", bufs=PS_BUFS, name=f"t5_{i}")
                for f in range(2):
                    tB = pspool.tile([128, 1024], F32, tag="ps", bufs=PS_BUFS, name=f"tB_{i}_{f}")
                    mm_chunk(tB[:, 0:OUT], f, 2)
                    mm_chunk(tB[:, 512:512 + OUT], f, 3)
                    copy(
                        STG[:, (f * 4 + 2) * OUT:(f * 4 + 4) * OUT].rearrange(
                            "p (b k) -> p b k", b=2),
                        tB[:].rearrange("p (b x) -> p b x", b=2)[:, :, 0:OUT],
                    )
                mm_chunk(t5[:, 0:OUT], 0, 4)
                mm_chunk(t5[:, 512:512 + OUT], 1, 4)
                copy(
                    STG[:, 8 * OUT:10 * OUT].rearrange("p (b k) -> p b k", b=2),
                    t5[:].rearrange("p (b x) -> p b x", b=2)[:, :, 0:OUT],
                )

                nc.gpsimd.dma_start(out=y_out[i], in_=STG[:])
    nc.finalize()
    return nc


def _get_module() -> bass.Bass:
    global _MODULE
    if _MODULE is None:
        _MODULE = _build_module()
    return _MODULE


def _make_in_maps(x: np.ndarray) -> list:
    imgs = x.reshape(N_CORES * IMGS, H, W).astype(NPBF16)
    return [
        {"x": imgs[k * IMGS:(k + 1) * IMGS], "w1": _W1, "w2": _W2}
        for k in range(N_CORES)
    ]


def _unpack(y: np.ndarray, B: int, C: int) -> np.ndarray:
    """y: [n_imgs, 128, 10*258] bf16 -> [B, 4C, 258, 258] fp32."""
    n = y.shape[0]
    y = y.astype(np.float32).reshape(n, 128, 10, OUT)
    full = np.empty((n, 4, OUT, OUT), np.float32)
    for b, (f, Ck) in enumerate(STG_BLOCKS):
        blk = y[:, :, b, :]  # [n, 128(g,kw), 258(kh)]
        for g in range(2):
            s = f + 2 * g
            sel = blk[:, g * 64:(g + 1) * 64, :]
            kws = KWSTART[Ck] + np.arange(64)
            if Ck == 3:
                pass  # kw 192..255, all valid
            elif Ck == 4:
                sel = sel[:, 62:, :]  # only kw 256, 257
                kws = kws[62:]
            full[:, s, :, kws[0]:kws[-1] + 1] = sel.transpose(0, 2, 1)
    return np.ascontiguousarray(full.reshape(B, 4 * C, OUT, OUT))


def kernel(**inputs) -> np.ndarray:
    x = np.asarray(inputs["x"], dtype=np.float32)
    B, C, Hx, Wx = x.shape
    assert (Hx, Wx) == (H, W) and B * C == N_CORES * IMGS

    nc = _get_module()
    res = run_bass_kernel_spmd(nc, _make_in_maps(x), list(range(N_CORES))).results
    y = np.concatenate([res[k]["y"] for k in range(N_CORES)], axis=0)
    return _unpack(y, B, C)


# revision 10
# speedup vs baseline: 1.0592x; 1.0452x over previous
"""Trainium2 Bass kernel for 2D single-level DWT (coif1, symmetric padding).

Input  x: (4, 64, 512, 512) fp32
Output  : (4, 256, 258, 258) fp32  -- per input channel: [cA, cH, cV, cD]

v2 design (bf16, banded half-blocks, SWDGE stores):
  pass 1 (contract rows r): r is split into half-blocks h in {0,1} of 256
    contiguous rows, each loaded as partition p <- rows (256h + 2p + j),
    j in {0,1} -- every DMA descriptor is 2 contiguous DRAM rows (2 KB).
    The 6-tap band of R_f means half-block h only feeds kh in
    [128h, 128h+130), so each matmul streams only 130 columns; the 2-col
    overlap accumulates via PSUM has_written semantics.
      Yt_f[c, kh] = sum_r X[r, c] R_f[kh, r]
  pass 2 (contract cols c): output rows (g, kw) are packed in uniform
    128-partition chunks of 64 kw x {lo,hi}: chunks start at kw =
    0, 64, 128, 192, 194 (the last overlaps; host keeps only kw 256-257
    from it).  Each chunk's band covers at most 2 c-blocks of 128 -> 1-2
    accumulating matmuls of 258 columns.
      O_{f,g}[kw, kh] = sum_c R_g[kw, c] Yt_f[c, kh]
  All matmuls bf16 (1 cycle/row at any free size, FWL weight loads).
  Loads and stores ride the gpsimd SWDGE ring: HWDGE stores with <128
  partitions serialize onto 2 of 16 SDMA engines; SWDGE spreads all 16.
  Output leaves packed ([i, p, block, kh], bf16); the host unpacks.
"""

import os
import sys

for _p in ("/opt/trn_rl_repo", "/opt/pypackages"):
    if _p not in sys.path:
        sys.path.append(_p)

os.environ.setdefault("JAX_COMPILATION_CACHE_DIR", "/tmp/jax_comp_cache")
os.environ.setdefault("JAX_PERSISTENT_CACHE_MIN_COMPILE_TIME_SECS", "10")

import numpy as np
import ml_dtypes

import concourse.bass as bass
import concourse.bacc as bacc
import concourse.mybir as mybir
from concourse.bass_utils import run_bass_kernel_spmd
from concourse.tile import TileContext

N_CORES = 8
H = W = 512
OUT = 258
IMGS = 32  # images per core (4*64/8)
F32 = mybir.dt.float32
BF16 = mybir.dt.bfloat16
NPBF16 = ml_dtypes.bfloat16

# pywt coif1 decomposition filters, flipped to correlation form
DEC_LO = np.array([-0.01565572813546454, -0.0727326195128539, 0.38486484686420286,
                   0.8525720202122554, 0.3378976624578092, -0.0727326195128539])
DEC_HI = np.array([0.0727326195128539, 0.3378976624578092, -0.8525720202122554,
                   0.38486484686420286, 0.0727326195128539, -0.01565572813546454])
FLEN = 6
PAD = 4
LO_F = DEC_LO[::-1]
HI_F = DEC_HI[::-1]

# pass-2 packed chunks: 64 kw starting at KWSTART[C]; CHUNK_CC[C] = c-blocks
KWSTART = [0, 64, 128, 192, 194]
CHUNK_CC = [[0], [0, 1], [1, 2], [2, 3], [3]]
W2_SLICES = [(C, cc) for C in range(5) for cc in CHUNK_CC[C]]  # 8 slices
# STG block order = device copy order (tA_f0, tB_f0, tA_f1, tB_f1, t5)
STG_BLOCKS = [(0, 0), (0, 1), (0, 2), (0, 3),
              (1, 0), (1, 1), (1, 2), (1, 3),
              (0, 4), (1, 4)]


def _build_R(filt: np.ndarray, n: int = W) -> np.ndarray:
    """Banded [258, 512] operator: out[k] = sum_j filt[j] * x[sym(2k + j - PAD)]."""
    out_len = (n + FLEN - 1) // 2

    def sym(i: int) -> int:
        while i < 0 or i >= n:
            if i < 0:
                i = -i - 1
            if i >= n:
                i = 2 * n - 1 - i
        return i

    R = np.zeros((out_len, n), dtype=np.float64)
    for k in range(out_len):
        for j in range(FLEN):
            R[k, sym(2 * k + j - PAD)] += filt[j]
    return R


_R = [_build_R(LO_F), _build_R(HI_F)]


def _build_w1() -> np.ndarray:
    """w1[p, ((f*2+h)*2+j)*130 + t] = R_f[128h + t, 256h + 2p + j]."""
    w = np.zeros((128, 8, 130), np.float64)
    for f in range(2):
        for h in range(2):
            for j in range(2):
                rows = 256 * h + 2 * np.arange(128) + j
                khs = 128 * h + np.arange(130)
                w[:, (f * 2 + h) * 2 + j, :] = _R[f][np.ix_(khs, rows)].T
    return w.reshape(128, 8 * 130).astype(NPBF16)


def _build_w2() -> np.ndarray:
    """w2[p, s*128 + u] for slice s=(C, cc): R_{u//64}[KWSTART[C] + u%64, 128cc + p]."""
    cols = []
    for C, cc in W2_SLICES:
        w = np.zeros((128, 128), np.float64)
        for u in range(128):
            g, kwo = divmod(u, 64)
            w[:, u] = _R[g][KWSTART[C] + kwo, cc * 128:(cc + 1) * 128]
        cols.append(w)
    return np.concatenate(cols, axis=1).astype(NPBF16)


_W = np.concatenate([_build_w1(), _build_w2()], axis=1)  # [128, 1040+1024]
_MODULE = None
PS_BUFS = 4
X_BUFS = 4
YT_BUFS = 2
STG_BUFS = 2


def _build_module() -> bass.Bass:
    nc = bacc.Bacc("TRN2", target_bir_lowering=False, debug=False)
    x_in = nc.declare_dram_parameter("x", [IMGS, H, W], BF16, isOutput=False)
    w_in = nc.declare_dram_parameter("w", [128, 2064], BF16, isOutput=False)
    y_out = nc.declare_dram_parameter("y", [IMGS, 128, 8 * OUT], BF16, isOutput=True)
    yt_out = nc.declare_dram_parameter("yt", [IMGS, 4, 2 * OUT], BF16, isOutput=True)

    with TileContext(nc) as tc:
        with (
            tc.tile_pool(name="wpool", bufs=1) as wpool,
            tc.tile_pool(name="xpool", bufs=X_BUFS) as xpool,
            tc.tile_pool(name="ypool", bufs=YT_BUFS) as ypool,
            tc.tile_pool(name="spool", bufs=STG_BUFS) as spool,
            tc.tile_pool(name="psum", bufs=2, space="PSUM") as pspool,
        ):
            # Prologue: weights on the sync ring (small, fast HWDGE gen) while
            # the first image's two row-halves generate in parallel on the
            # scalar HWDGE ring and the gpsimd SWDGE ring.
            Wt = wpool.tile([128, 2064], BF16)
            nc.sync.dma_start(out=Wt[:], in_=w_in[:])
            Wt1 = Wt[:, 0:1040]
            Wt2 = Wt[:, 1040:2064]
            X0 = xpool.tile([128, 2, 1024], BF16, tag="X", name="X_0")
            x0v = x_in[0].rearrange("(h p j) c -> p h (j c)", h=2, j=2)
            nc.scalar.dma_start(out=X0[:, 0], in_=x0v[:, 0])
            nc.gpsimd.dma_start(out=X0[:, 1], in_=x0v[:, 1])

            # Tiny PE op consuming both weight DMAs so later matmuls depend
            # on them via PE program order (Matmult carries one sync wait).
            warm = pspool.tile([128, 1024], F32, tag="ps", bufs=PS_BUFS,
                               name="warm")
            nc.tensor.matmul(warm[0:1, 0:OUT], lhsT=Wt2[:, 0:1],
                             rhs=Wt1[:, 0:OUT], start=True, stop=True)

            ev = 0

            def copy(dst, src):
                nonlocal ev
                if ev % 2 == 0:
                    nc.scalar.copy(out=dst, in_=src)
                else:
                    nc.vector.tensor_copy(out=dst, in_=src)
                ev += 1

            def load_x(i):
                # X[p, h, j*512 + c] = x[i, 256h + 2p + j, c]
                X = xpool.tile([128, 2, 1024], BF16, tag="X", name=f"X_{i}")
                nc.gpsimd.dma_start(
                    out=X[:],
                    in_=x_in[i].rearrange("(h p j) c -> p h (j c)", h=2, j=2),
                )
                return X

            Xnext = X0
            for i in range(IMGS):
                Xr = Xnext[:]
                Yt = ypool.tile([128, 8 * OUT], BF16, tag="Yt", name=f"Yt_{i}")

                # pass 1: Yt[p, (f*4+cc)*258 + kh] = Yt_f[c = 128cc + p, kh]
                for ccp in range(2):  # cc pair (2*ccp, 2*ccp+1)
                    ps1 = [pspool.tile([128, 1024], F32, tag="ps", bufs=PS_BUFS,
                                       name=f"ps1_{i}_{ccp}_{f}")
                           for f in range(2)]
                    for ci in range(2):
                        cc = 2 * ccp + ci
                        for h in range(2):
                            for j in range(2):
                                lhsT = Xr[:, h, j * 512 + cc * 128:
                                          j * 512 + cc * 128 + 128]
                                for f in range(2):
                                    nc.tensor.matmul(
                                        ps1[f][:, ci * 512 + 128 * h:
                                               ci * 512 + 128 * h + 130],
                                        lhsT=lhsT,
                                        rhs=Wt1[:, ((f * 2 + h) * 2 + j) * 130:
                                                ((f * 2 + h) * 2 + j + 1) * 130],
                                        start=(h == 0 and j == 0),
                                        stop=(h == 1 and j == 1),
                                    )
                    for f in range(2):
                        src = ps1[f][:].rearrange("p (b x) -> p b x", b=2)[:, :, 0:OUT]
                        dst = Yt[:, (f * 4 + 2 * ccp) * OUT:
                                 (f * 4 + 2 * ccp + 2) * OUT].rearrange(
                                     "p (b k) -> p b k", b=2)
                        copy(dst, src)

                # prefetch next image's input
                if i + 1 < IMGS:
                    Xnext = load_x(i + 1)

                # pass 2: STG blocks per STG_BLOCKS order
                STG = spool.tile([128, 8 * OUT], BF16, tag="STG", name=f"STG_{i}")
                STGT = spool.tile([128, 2 * OUT], BF16, tag="STGT", name=f"STGT_{i}")

                def mm_chunk(dst_ap, f, C):
                    ccs = CHUNK_CC[C]
                    for a, cc in enumerate(ccs):
                        s_idx = W2_SLICES.index((C, cc))
                        nc.tensor.matmul(
                            dst_ap,
                            lhsT=Wt2[:, s_idx * 128:(s_idx + 1) * 128],
                            rhs=Yt[:, (f * 4 + cc) * OUT:(f * 4 + cc + 1) * OUT],
                            start=(a == 0),
                            stop=(a == len(ccs) - 1),
                        )

                for f in range(2):
                    tA = pspool.tile([128, 1024], F32, tag="ps", bufs=PS_BUFS, name=f"tA_{i}_{f}")
                    mm_chunk(tA[:, 0:OUT], f, 0)
                    mm_chunk(tA[:, 512:512 + OUT], f, 1)
                    copy(
                        STG[:, (f * 4) * OUT:(f * 4 + 2) * OUT].rearrange(
                            "p (b k) -> p b k", b=2),
                        tA[:].rearrange("p (b x) -> p b x", b=2)[:, :, 0:OUT],
                    )
                t5 = pspool.tile([128, 1024], F32, tag="ps", bufs=PS_BUFS, name=f"t5_{i}")
                for f in range(2):
                    tB = pspool.tile([128, 1024], F32, tag="ps", bufs=PS_BUFS, name=f"tB_{i}_{f}")
                    mm_chunk(tB[:, 0:OUT], f, 2)
                    mm_chunk(tB[:, 512:512 + OUT], f, 3)
                    copy(
                        STG[:, (f * 4 + 2) * OUT:(f * 4 + 4) * OUT].rearrange(
                            "p (b k) -> p b k", b=2),
                        tB[:].rearrange("p (b x) -> p b x", b=2)[:, :, 0:OUT],
                    )
                mm_chunk(t5[:, 0:OUT], 0, 4)
                mm_chunk(t5[:, 512:512 + OUT], 1, 4)
                copy(
                    STGT[:].rearrange("p (b k) -> p b k", b=2),
                    t5[:].rearrange("p (b x) -> p b x", b=2)[:, :, 0:OUT],
                )

                nc.gpsimd.dma_start(out=y_out[i], in_=STG[:])
                # kw 256/257 tail: 4 meaningful partitions, tiny HWDGE stores
                nc.sync.dma_start(out=yt_out[i, 0:2], in_=STGT[62:64, :])
                nc.sync.dma_start(out=yt_out[i, 2:4], in_=STGT[126:128, :])
    nc.finalize()
    return nc


def _get_module() -> bass.Bass:
    global _MODULE
    if _MODULE is None:
        _MODULE = _build_module()
    return _MODULE


def _make_in_maps(x: np.ndarray) -> list:
    imgs = x.reshape(N_CORES * IMGS, H, W).astype(NPBF16)
    return [
        {"x": imgs[k * IMGS:(k + 1) * IMGS], "w": _W}
        for k in range(N_CORES)
    ]


def _unpack(y: np.ndarray, yt: np.ndarray, B: int, C: int) -> np.ndarray:
    """y: [n, 128, 8*258], yt: [n, 4, 2*258] bf16 -> [B, 4C, 258, 258] fp32."""
    n = y.shape[0]
    y = y.astype(np.float32).reshape(n, 128, 8, OUT)
    yt = yt.astype(np.float32).reshape(n, 4, 2, OUT)
    full = np.empty((n, 4, OUT, OUT), np.float32)
    for b, (f, Ck) in enumerate(STG_BLOCKS[:8]):
        blk = y[:, :, b, :]  # [n, 128(g,kw), 258(kh)]
        for g in range(2):
            s = f + 2 * g
            sel = blk[:, g * 64:(g + 1) * 64, :]
            kws = KWSTART[Ck] + np.arange(64)
            full[:, s, :, kws[0]:kws[-1] + 1] = sel.transpose(0, 2, 1)
    # tail: yt[:, 2g + r, f*258:(f+1)*258] = O_{f,g}[kw=256+r, :]
    for f in range(2):
        for g in range(2):
            for r in range(2):
                full[:, f + 2 * g, :, 256 + r] = yt[:, 2 * g + r, f, :]
    return np.ascontiguousarray(full.reshape(B, 4 * C, OUT, OUT))


def kernel(**inputs) -> np.ndarray:
    x = np.asarray(inputs["x"], dtype=np.float32)
    B, C, Hx, Wx = x.shape
    assert (Hx, Wx) == (H, W) and B * C == N_CORES * IMGS

    nc = _get_module()
    res = run_bass_kernel_spmd(nc, _make_in_maps(x), list(range(N_CORES))).results
    y = np.concatenate([res[k]["y"] for k in range(N_CORES)], axis=0)
    yt = np.concatenate([res[k]["yt"] for k in range(N_CORES)], axis=0)
    return _unpack(y, yt, B, C)


# revision 11
# speedup vs baseline: 1.0697x; 1.0099x over previous
"""Trainium2 Bass kernel for 2D single-level DWT (coif1, symmetric padding).

Input  x: (4, 64, 512, 512) fp32
Output  : (4, 256, 258, 258) fp32  -- per input channel: [cA, cH, cV, cD]

v2 design (bf16, banded half-blocks, SWDGE stores):
  pass 1 (contract rows r): r is split into half-blocks h in {0,1} of 256
    contiguous rows, each loaded as partition p <- rows (256h + 2p + j),
    j in {0,1} -- every DMA descriptor is 2 contiguous DRAM rows (2 KB).
    The 6-tap band of R_f means half-block h only feeds kh in
    [128h, 128h+130), so each matmul streams only 130 columns; the 2-col
    overlap accumulates via PSUM has_written semantics.
      Yt_f[c, kh] = sum_r X[r, c] R_f[kh, r]
  pass 2 (contract cols c): output rows (g, kw) are packed in uniform
    128-partition chunks of 64 kw x {lo,hi}: chunks start at kw =
    0, 64, 128, 192, 194 (the last overlaps; host keeps only kw 256-257
    from it).  Each chunk's band covers at most 2 c-blocks of 128 -> 1-2
    accumulating matmuls of 258 columns.
      O_{f,g}[kw, kh] = sum_c R_g[kw, c] Yt_f[c, kh]
  All matmuls bf16 (1 cycle/row at any free size, FWL weight loads).
  Loads and stores ride the gpsimd SWDGE ring: HWDGE stores with <128
  partitions serialize onto 2 of 16 SDMA engines; SWDGE spreads all 16.
  Output leaves packed ([i, p, block, kh], bf16); the host unpacks.
"""

import os
import sys

for _p in ("/opt/trn_rl_repo", "/opt/pypackages"):
    if _p not in sys.path:
        sys.path.append(_p)

os.environ.setdefault("JAX_COMPILATION_CACHE_DIR", "/tmp/jax_comp_cache")
os.environ.setdefault("JAX_PERSISTENT_CACHE_MIN_COMPILE_TIME_SECS", "10")

import numpy as np
import ml_dtypes

import concourse.bass as bass
import concourse.bacc as bacc
import concourse.mybir as mybir
from concourse.bass_utils import run_bass_kernel_spmd
from concourse.tile import TileContext

N_CORES = 8
H = W = 512
OUT = 258
IMGS = 32  # images per core (4*64/8)
F32 = mybir.dt.float32
BF16 = mybir.dt.bfloat16
NPBF16 = ml_dtypes.bfloat16

# pywt coif1 decomposition filters, flipped to correlation form
DEC_LO = np.array([-0.01565572813546454, -0.0727326195128539, 0.38486484686420286,
                   0.8525720202122554, 0.3378976624578092, -0.0727326195128539])
DEC_HI = np.array([0.0727326195128539, 0.3378976624578092, -0.8525720202122554,
                   0.38486484686420286, 0.0727326195128539, -0.01565572813546454])
FLEN = 6
PAD = 4
LO_F = DEC_LO[::-1]
HI_F = DEC_HI[::-1]

# pass-2 packed chunks: 64 kw starting at KWSTART[C]; CHUNK_CC[C] = c-blocks
KWSTART = [0, 64, 128, 192, 194]
CHUNK_CC = [[0], [0, 1], [1, 2], [2, 3], [3]]
W2_SLICES = [(C, cc) for C in range(5) for cc in CHUNK_CC[C]]  # 8 slices
# STG block order = device copy order (tA_f0, tB_f0, tA_f1, tB_f1, t5)
STG_BLOCKS = [(0, 0), (0, 1), (0, 2), (0, 3),
              (1, 0), (1, 1), (1, 2), (1, 3),
              (0, 4), (1, 4)]


def _build_R(filt: np.ndarray, n: int = W) -> np.ndarray:
    """Banded [258, 512] operator: out[k] = sum_j filt[j] * x[sym(2k + j - PAD)]."""
    out_len = (n + FLEN - 1) // 2

    def sym(i: int) -> int:
        while i < 0 or i >= n:
            if i < 0:
                i = -i - 1
            if i >= n:
                i = 2 * n - 1 - i
        return i

    R = np.zeros((out_len, n), dtype=np.float64)
    for k in range(out_len):
        for j in range(FLEN):
            R[k, sym(2 * k + j - PAD)] += filt[j]
    return R


_R = [_build_R(LO_F), _build_R(HI_F)]


def _build_w1() -> np.ndarray:
    """w1[p, ((f*2+h)*2+j)*130 + t] = R_f[128h + t, 256h + 2p + j]."""
    w = np.zeros((128, 8, 130), np.float64)
    for f in range(2):
        for h in range(2):
            for j in range(2):
                rows = 256 * h + 2 * np.arange(128) + j
                khs = 128 * h + np.arange(130)
                w[:, (f * 2 + h) * 2 + j, :] = _R[f][np.ix_(khs, rows)].T
    return w.reshape(128, 8 * 130).astype(NPBF16)


def _build_w2() -> np.ndarray:
    """w2[p, s*128 + u] for slice s=(C, cc): R_{u//64}[KWSTART[C] + u%64, 128cc + p]."""
    cols = []
    for C, cc in W2_SLICES:
        w = np.zeros((128, 128), np.float64)
        for u in range(128):
            g, kwo = divmod(u, 64)
            w[:, u] = _R[g][KWSTART[C] + kwo, cc * 128:(cc + 1) * 128]
        cols.append(w)
    return np.concatenate(cols, axis=1).astype(NPBF16)


_W = np.concatenate([_build_w1(), _build_w2()], axis=1)  # [128, 1040+1024]
_MODULE = None
PS_BUFS = 4
X_BUFS = 4
YT_BUFS = 2
STG_BUFS = 2


def _build_module() -> bass.Bass:
    nc = bacc.Bacc("TRN2", target_bir_lowering=False, debug=False)
    x_in = nc.declare_dram_parameter("x", [IMGS, H, W], BF16, isOutput=False)
    w_in = nc.declare_dram_parameter("w", [128, 2064], BF16, isOutput=False)
    y_out = nc.declare_dram_parameter("y", [IMGS, 128, 8 * OUT], BF16, isOutput=True)
    yt_out = nc.declare_dram_parameter("yt", [IMGS, 4, 2 * OUT], BF16, isOutput=True)

    with TileContext(nc) as tc:
        with (
            tc.tile_pool(name="wpool", bufs=1) as wpool,
            tc.tile_pool(name="xpool", bufs=X_BUFS) as xpool,
            tc.tile_pool(name="ypool", bufs=YT_BUFS) as ypool,
            tc.tile_pool(name="spool", bufs=STG_BUFS) as spool,
            tc.tile_pool(name="psum", bufs=2, space="PSUM") as pspool,
        ):
            # Prologue: weights + first row-half on the SWDGE ring (fast
            # descriptor gen); second row-half in parallel on the scalar
            # HWDGE ring.  Pass-1 MMs consume h0 before h1, so image 0's
            # compute starts as soon as w + h0 land.
            Wt = wpool.tile([128, 2064], BF16)
            nc.gpsimd.dma_start(out=Wt[:], in_=w_in[:])
            Wt1 = Wt[:, 0:1040]
            Wt2 = Wt[:, 1040:2064]
            X0 = xpool.tile([128, 2, 1024], BF16, tag="X", name="X_0")
            x0v = x_in[0].rearrange("(h p j) c -> p h (j c)", h=2, j=2)
            nc.gpsimd.dma_start(out=X0[:, 0], in_=x0v[:, 0])
            nc.scalar.dma_start(out=X0[:, 1], in_=x0v[:, 1])

            # Tiny PE op consuming both weight DMAs so later matmuls depend
            # on them via PE program order (Matmult carries one sync wait).
            warm = pspool.tile([128, 1024], F32, tag="ps", bufs=PS_BUFS,
                               name="warm")
            nc.tensor.matmul(warm[0:1, 0:OUT], lhsT=Wt2[:, 0:1],
                             rhs=Wt1[:, 0:OUT], start=True, stop=True)

            ev = 0

            def copy(dst, src):
                nonlocal ev
                if ev % 2 == 0:
                    nc.scalar.copy(out=dst, in_=src)
                else:
                    nc.vector.tensor_copy(out=dst, in_=src)
                ev += 1

            def load_x(i):
                # X[p, h, j*512 + c] = x[i, 256h + 2p + j, c]
                X = xpool.tile([128, 2, 1024], BF16, tag="X", name=f"X_{i}")
                nc.gpsimd.dma_start(
                    out=X[:],
                    in_=x_in[i].rearrange("(h p j) c -> p h (j c)", h=2, j=2),
                )
                return X

            Xnext = X0
            for i in range(IMGS):
                Xr = Xnext[:]
                Yt = ypool.tile([128, 8 * OUT], BF16, tag="Yt", name=f"Yt_{i}")

                # pass 1: Yt[p, (f*4+cc)*258 + kh] = Yt_f[c = 128cc + p, kh]
                for ccp in range(2):  # cc pair (2*ccp, 2*ccp+1)
                    ps1 = [pspool.tile([128, 1024], F32, tag="ps", bufs=PS_BUFS,
                                       name=f"ps1_{i}_{ccp}_{f}")
                           for f in range(2)]
                    for ci in range(2):
                        cc = 2 * ccp + ci
                        for h in range(2):
                            for j in range(2):
                                lhsT = Xr[:, h, j * 512 + cc * 128:
                                          j * 512 + cc * 128 + 128]
                                for f in range(2):
                                    nc.tensor.matmul(
                                        ps1[f][:, ci * 512 + 128 * h:
                                               ci * 512 + 128 * h + 130],
                                        lhsT=lhsT,
                                        rhs=Wt1[:, ((f * 2 + h) * 2 + j) * 130:
                                                ((f * 2 + h) * 2 + j + 1) * 130],
                                        start=(h == 0 and j == 0),
                                        stop=(h == 1 and j == 1),
                                    )
                    for f in range(2):
                        src = ps1[f][:].rearrange("p (b x) -> p b x", b=2)[:, :, 0:OUT]
                        dst = Yt[:, (f * 4 + 2 * ccp) * OUT:
                                 (f * 4 + 2 * ccp + 2) * OUT].rearrange(
                                     "p (b k) -> p b k", b=2)
                        copy(dst, src)

                # prefetch next image's input
                if i + 1 < IMGS:
                    Xnext = load_x(i + 1)

                # pass 2: STG blocks per STG_BLOCKS order
                STG = spool.tile([128, 8 * OUT], BF16, tag="STG", name=f"STG_{i}")
                STGT = spool.tile([128, 2 * OUT], BF16, tag="STGT", name=f"STGT_{i}")

                def mm_chunk(dst_ap, f, C):
                    ccs = CHUNK_CC[C]
                    for a, cc in enumerate(ccs):
                        s_idx = W2_SLICES.index((C, cc))
                        nc.tensor.matmul(
                            dst_ap,
                            lhsT=Wt2[:, s_idx * 128:(s_idx + 1) * 128],
                            rhs=Yt[:, (f * 4 + cc) * OUT:(f * 4 + cc + 1) * OUT],
                            start=(a == 0),
                            stop=(a == len(ccs) - 1),
                        )

                for f in range(2):
                    tA = pspool.tile([128, 1024], F32, tag="ps", bufs=PS_BUFS, name=f"tA_{i}_{f}")
                    mm_chunk(tA[:, 0:OUT], f, 0)
                    mm_chunk(tA[:, 512:512 + OUT], f, 1)
                    copy(
                        STG[:, (f * 4) * OUT:(f * 4 + 2) * OUT].rearrange(
                            "p (b k) -> p b k", b=2),
                        tA[:].rearrange("p (b x) -> p b x", b=2)[:, :, 0:OUT],
                    )
                t5 = pspool.tile([128, 1024], F32, tag="ps", bufs=PS_BUFS, name=f"t5_{i}")
                for f in range(2):
                    tB = pspool.tile([128, 1024], F32, tag="ps", bufs=PS_BUFS, name=f"tB_{i}_{f}")
                    mm_chunk(tB[:, 0:OUT], f, 2)
                    mm_chunk(tB[:, 512:512 + OUT], f, 3)
                    copy(
                        STG[:, (f * 4 + 2) * OUT:(f * 4 + 4) * OUT].rearrange(
                            "p (b k) -> p b k", b=2),
                        tB[:].rearrange("p (b x) -> p b x", b=2)[:, :, 0:OUT],
                    )
                mm_chunk(t5[:, 0:OUT], 0, 4)
                mm_chunk(t5[:, 512:512 + OUT], 1, 4)
                copy(
                    STGT[:].rearrange("p (b k) -> p b k", b=2),
                    t5[:].rearrange("p (b x) -> p b x", b=2)[:, :, 0:OUT],
                )

                if i >= IMGS - 2:
                    # drain the tail of the pipeline in halves so the final
                    # store's transfer starts before the last copies land
                    nc.gpsimd.dma_start(out=y_out[i, :, 0:4 * OUT],
                                        in_=STG[:, 0:4 * OUT])
                    nc.gpsimd.dma_start(out=y_out[i, :, 4 * OUT:8 * OUT],
                                        in_=STG[:, 4 * OUT:8 * OUT])
                else:
                    nc.gpsimd.dma_start(out=y_out[i], in_=STG[:])
                # kw 256/257 tail: 4 meaningful partitions, tiny HWDGE stores
                nc.sync.dma_start(out=yt_out[i, 0:2], in_=STGT[62:64, :])
                nc.sync.dma_start(out=yt_out[i, 2:4], in_=STGT[126:128, :])
    nc.finalize()
    return nc


def _get_module() -> bass.Bass:
    global _MODULE
    if _MODULE is None:
        _MODULE = _build_module()
    return _MODULE


def _make_in_maps(x: np.ndarray) -> list:
    imgs = x.reshape(N_CORES * IMGS, H, W).astype(NPBF16)
    return [
        {"x": imgs[k * IMGS:(k + 1) * IMGS], "w": _W}
        for k in range(N_CORES)
    ]


def _unpack(y: np.ndarray, yt: np.ndarray, B: int, C: int) -> np.ndarray:
    """y: [n, 128, 8*258], yt: [n, 4, 2*258] bf16 -> [B, 4C, 258, 258] fp32."""
    n = y.shape[0]
    y = y.astype(np.float32).reshape(n, 128, 8, OUT)
    yt = yt.astype(np.float32).reshape(n, 4, 2, OUT)
    full = np.empty((n, 4, OUT, OUT), np.float32)
    for b, (f, Ck) in enumerate(STG_BLOCKS[:8]):
        blk = y[:, :, b, :]  # [n, 128(g,kw), 258(kh)]
        for g in range(2):
            s = f + 2 * g
            sel = blk[:, g * 64:(g + 1) * 64, :]
            kws = KWSTART[Ck] + np.arange(64)
            full[:, s, :, kws[0]:kws[-1] + 1] = sel.transpose(0, 2, 1)
    # tail: yt[:, 2g + r, f*258:(f+1)*258] = O_{f,g}[kw=256+r, :]
    for f in range(2):
        for g in range(2):
            for r in range(2):
                full[:, f + 2 * g, :, 256 + r] = yt[:, 2 * g + r, f, :]
    return np.ascontiguousarray(full.reshape(B, 4 * C, OUT, OUT))


def kernel(**inputs) -> np.ndarray:
    x = np.asarray(inputs["x"], dtype=np.float32)
    B, C, Hx, Wx = x.shape
    assert (Hx, Wx) == (H, W) and B * C == N_CORES * IMGS

    nc = _get_module()
    res = run_bass_kernel_spmd(nc, _make_in_maps(x), list(range(N_CORES))).results
    y = np.concatenate([res[k]["y"] for k in range(N_CORES)], axis=0)
    yt = np.concatenate([res[k]["yt"] for k in range(N_CORES)], axis=0)
    return _unpack(y, yt, B, C)
